# revision 18
# baseline (speedup 1.0000x reference)
"""LoRA-MoE Linear kernel for Trainium2, 8-core SPMD.

Strategy (token-parallel, bf16 compute):
  - 8192 tokens sharded across 8 cores (1024 each). Host pre-lays-out every
    large operand so each DMA is contiguous per partition.
  - All large matmuls in bf16 (full PE rate); PSUM accumulates fp32.
  - Pass 1 (channel-major): tmp[er,tok] = A^T x and d[32,tok] = (G1-mean)^T x
    share one N=512 moving-x k-loop. The LayerNorm mean subtraction is folded
    into centered gate weights host-side, so PSUM directly yields d.
  - Routing stays expert-major [8,tok]: partition_all_reduce (GpSimd) for
    var / top-2 maxima; zero PE transposes.
  - Engine queues are strict FIFO per engine, so placement = program order:
      PE:     [pass1 + oc0 + oc1 interleaved, chasing the x DMA] | oc2 oc3 |
              gates | oc4..7 (unfused) | wbr | oc8..31 (lora fused into the
              PSUM accumulation) | lora tail for oc<8
      Vector: pass1 copies, LN, pre-collective routing, post-collective
              combine, tw (nothing after => can't block anything)
      Scalar: Sqrt, Sigmoid, ALL PSUM->SBUF output copies (ACT.Copy)
      GpSimd: partition reductions, counts-AllReduce + its DMAs
    The counts AllReduce finishes ~150-220us (mesh hop DMAs queue behind
    weight-slab traffic), so LoRA is only fused for oc>=8; oc<8 get a
    separate LoRA pass at the tail written to loraT, host adds.
"""

import numpy as np
import ml_dtypes

import concourse.bacc as bacc
import concourse.bass as bass
import concourse.bass_isa as bass_isa
import concourse.mybir as mybir
import concourse.tile as tile
from concourse.bass_utils import run_bass_kernel_spmd

F32 = mybir.dt.float32
BF16 = mybir.dt.bfloat16
AX = mybir.AxisListType
ALU = mybir.AluOpType
ACT = mybir.ActivationFunctionType
RED = bass_isa.ReduceOp
BF = ml_dtypes.bfloat16

B, S, IN, OUT = 4, 2048, 4096, 4096
E, K, R = 8, 2, 16
CAP_FACTOR = 3.0
ALPHA = 1.0 / R
LN_EPS = 1e-5
N_CORES = 8
N_TOK = B * S               # 8192
TPC = N_TOK // N_CORES      # 1024 tokens per core
G4E = 4 * E                 # 32 gate hidden
ER = E * R                  # 128
KT = IN // 128              # 32 contraction tiles
OC = OUT // 128             # 32 output column blocks
NEG = -1.0e30
CAPACITY = float(int(CAP_FACTOR * N_TOK / E))  # 3072
TH = TPC // 512             # 2 token halves of 512
FUSE_OC = 8                 # oc >= FUSE_OC get LoRA fused into main PSUM


def build_bass():
    nc = bacc.Bacc(
        "TRN2", target_bir_lowering=False, debug=False, num_devices=N_CORES
    )
    xp = nc.dram_tensor("xp", [128, KT * TPC], BF16, kind="ExternalInput")
    wp = nc.dram_tensor("wp", [OC, 128, KT * 128], BF16, kind="ExternalInput")
    ap_ = nc.dram_tensor("ap_", [128, KT * ER], BF16, kind="ExternalInput")
    g1p = nc.dram_tensor("g1p", [128, KT * G4E], BF16, kind="ExternalInput")
    bp = nc.dram_tensor("bp", [ER, OUT], BF16, kind="ExternalInput")
    g2p = nc.dram_tensor("g2p", [G4E, E], BF16, kind="ExternalInput")
    repp = nc.dram_tensor("repp", [E, ER], BF16, kind="ExternalInput")
    gb1c = nc.dram_tensor("gb1c", [G4E, 1], F32, kind="ExternalInput")
    gamc = nc.dram_tensor("gamc", [G4E, 1], F32, kind="ExternalInput")
    betc = nc.dram_tensor("betc", [G4E, 1], F32, kind="ExternalInput")
    gb2c = nc.dram_tensor("gb2c", [E, 1], F32, kind="ExternalInput")
    outT = nc.dram_tensor("outT", [OUT, TPC], F32, kind="ExternalOutput")
    loraT = nc.dram_tensor("loraT", [FUSE_OC * 128, TPC], F32, kind="ExternalOutput")

    with tile.TileContext(nc) as tc:
        with (
            tc.tile_pool(name="big", bufs=1) as big,
            tc.tile_pool(name="rt", bufs=1) as rt,
            tc.tile_pool(name="wsl", bufs=3) as wsp,
            tc.tile_pool(name="outp", bufs=4) as op_,
            tc.tile_pool(name="ps_a", bufs=1, space="PSUM") as psa,
            tc.tile_pool(name="ps_c", bufs=2, space="PSUM") as psc,
            tc.tile_pool(name="ps_m", bufs=4, space="PSUM") as psm,
            tc.tile_pool(name="dram", bufs=1, space="DRAM") as dp,
        ):
            # ---- resident loads (x first, fine-grained across queues) -----
            # Single-queue DMA sustains only ~30 GB/s: split x into (k, th)
            # half-tile chunks and a into 4-k chunks so the 16 queues all
            # pull the pass-1 critical path in parallel.
            xT_sb = big.tile([128, KT, TPC], BF16)
            a_sb = big.tile([128, KT, ER], BF16)
            for k in range(KT):
                if k < 4:
                    for th in range(TH):
                        nc.sync.dma_start(
                            xT_sb[:, k, th * 512 : (th + 1) * 512],
                            xp.ap()[
                                :, k * TPC + th * 512 : k * TPC + (th + 1) * 512
                            ],
                        )
                else:
                    nc.sync.dma_start(
                        xT_sb[:, k], xp.ap()[:, k * TPC : (k + 1) * TPC]
                    )
                if k % 8 == 0:
                    c = k // 8
                    nc.sync.dma_start(
                        a_sb[:, 8 * c : 8 * c + 8],
                        ap_.ap()[:, 8 * c * ER : (8 * c + 8) * ER].rearrange(
                            "p (k e) -> p k e", e=ER
                        ),
                    )
            g1_sb = big.tile([128, KT, G4E], BF16)
            nc.sync.dma_start(
                g1_sb, g1p.ap().rearrange("p (k g) -> p k g", g=G4E)
            )
            g2_sb = big.tile([G4E, E], BF16)
            nc.sync.dma_start(g2_sb, g2p.ap())
            repp_sb = big.tile([E, ER], BF16)
            nc.sync.dma_start(repp_sb, repp.ap())
            gb1c_sb = big.tile([G4E, 1], F32)
            nc.sync.dma_start(gb1c_sb, gb1c.ap())
            gamc_sb = big.tile([G4E, 1], F32)
            nc.sync.dma_start(gamc_sb, gamc.ap())
            betc_sb = big.tile([G4E, 1], F32)
            nc.sync.dma_start(betc_sb, betc.ap())
            gb2c_sb = big.tile([E, 1], F32)
            nc.sync.dma_start(gb2c_sb, gb2c.ap())
            eps_sb = big.tile([G4E, 1], F32)
            nc.vector.memset(eps_sb, LN_EPS)

            # ---- pass 1 + oc0/oc1 main blocks, one x-chasing k-loop -------
            # 8 MMs per k-tile (~2.2us) vs ~0.7us DMA arrival: PE saturates
            # from the second tile and the whole x load hides under compute.
            tmp_ps = [psa.tile([128, 512], F32, name=f"tmp{t}") for t in range(TH)]
            hT_ps = [
                psc.tile([G4E, 512], F32, tag="sm", name=f"hT{t}") for t in range(TH)
            ]
            wsl01 = []
            pos01 = []
            for oc in range(2):
                wsl = wsp.tile([128, KT, 128], BF16, tag="wsl")
                for q in range(4):
                    nc.sync.dma_start(
                        wsl[:, 8 * q : 8 * q + 8],
                        wp.ap()[oc][:, 8 * q * 128 : (8 * q + 8) * 128].rearrange(
                            "p (k c) -> p k c", c=128
                        ),
                    )
                wsl01.append(wsl)
                pos01.append(
                    [
                        psm.tile([128, 512], F32, tag="po", name=f"po{oc}_{t}")
                        for t in range(TH)
                    ]
                )
            for k in range(KT):
                first, last = k == 0, k == KT - 1
                for th in range(TH):
                    nc.tensor.matmul(
                        tmp_ps[th], a_sb[:, k],
                        xT_sb[:, k, th * 512 : (th + 1) * 512],
                        start=first, stop=last,
                    )
                for th in range(TH):
                    nc.tensor.matmul(
                        hT_ps[th], g1_sb[:, k],
                        xT_sb[:, k, th * 512 : (th + 1) * 512],
                        start=first, stop=last,
                    )
                for oc in range(2):
                    for th in range(TH):
                        nc.tensor.matmul(
                            pos01[oc][th], wsl01[oc][:, k],
                            xT_sb[:, k, th * 512 : (th + 1) * 512],
                            start=first, stop=last,
                        )
            tmp_sb = big.tile([128, TPC], F32)
            d_sb = big.tile([G4E, TPC], F32)
            for th in range(TH):
                sl = slice(th * 512, (th + 1) * 512)
                nc.vector.tensor_copy(tmp_sb[:, sl], tmp_ps[th])
                # d = (G1-centered)^T x + (gb1 - mean(gb1))  [host-folded]
                nc.vector.tensor_scalar(
                    out=d_sb[:, sl], in0=hT_ps[th], scalar1=gb1c_sb,
                    scalar2=None, op0=ALU.add,
                )
            for oc in range(2):
                for th in range(TH):
                    sl = slice(th * 512, (th + 1) * 512)
                    osb = op_.tile([128, 512], F32, tag="osb")
                    nc.scalar.activation(osb, pos01[oc][th], ACT.Copy)
                    nc.sync.dma_start(
                        outT.ap()[oc * 128 : (oc + 1) * 128, sl], osb
                    )

            # b only feeds the (late) LoRA-B matmuls: load it after the
            # x-critical window so it doesn't compete for DMA queues.
            b_sb = big.tile([ER, OUT], BF16)
            for hh in range(2):
                nc.sync.dma_start(
                    b_sb[:, hh * 2048 : (hh + 1) * 2048],
                    bp.ap()[:, hh * 2048 : (hh + 1) * 2048],
                )

            def main_oc(oc, fused):
                wsl = wsp.tile([128, KT, 128], BF16, tag="wsl")
                nc.sync.dma_start(
                    wsl, wp.ap()[oc].rearrange("p (k c) -> p k c", c=128)
                )
                pos = [
                    psm.tile([128, 512], F32, tag="po", name=f"po{oc}_{t}")
                    for t in range(TH)
                ]
                for k in range(KT):
                    for th in range(TH):
                        nc.tensor.matmul(
                            pos[th], wsl[:, k],
                            xT_sb[:, k, th * 512 : (th + 1) * 512],
                            start=(k == 0),
                            stop=(not fused and k == KT - 1),
                        )
                for th in range(TH):
                    sl = slice(th * 512, (th + 1) * 512)
                    if fused:
                        nc.tensor.matmul(
                            pos[th], b_sb[:, oc * 128 : (oc + 1) * 128],
                            tw_bf[:, sl], start=False, stop=True,
                        )
                    osb = op_.tile([128, 512], F32, tag="osb")
                    nc.scalar.activation(osb, pos[th], ACT.Copy)
                    nc.sync.dma_start(
                        outT.ap()[oc * 128 : (oc + 1) * 128, sl], osb
                    )

            tw_bf = big.tile([128, TPC], BF16)
            main_oc(2, False)
            main_oc(3, False)

            # ---- LayerNorm tail + gate logits -----------------------------
            sq = rt.tile([G4E, TPC], F32, tag="sq")
            nc.vector.tensor_tensor(out=sq, in0=d_sb, in1=d_sb, op=ALU.mult)
            varb = rt.tile([G4E, TPC], F32, tag="varb")
            nc.gpsimd.partition_all_reduce(varb, sq, channels=G4E, reduce_op=RED.add)
            rstd = rt.tile([G4E, TPC], F32, tag="rstd")
            nc.scalar.activation(
                rstd, varb, ACT.Sqrt, bias=eps_sb[:, :], scale=1.0 / G4E
            )
            nc.vector.reciprocal(rstd, rstd)
            nc.vector.tensor_tensor(out=d_sb, in0=d_sb, in1=rstd, op=ALU.mult)
            nc.vector.tensor_scalar(
                out=d_sb, in0=d_sb, scalar1=gamc_sb, scalar2=None, op0=ALU.mult
            )
            nc.vector.tensor_scalar(
                out=d_sb, in0=d_sb, scalar1=betc_sb, scalar2=None, op0=ALU.add
            )
            hn_bf = big.tile([G4E, TPC], BF16)
            nc.vector.tensor_scalar_max(hn_bf, d_sb, 0.0)

            gates = rt.tile([E, TPC], F32, tag="gates")
            for th in range(TH):
                sl = slice(th * 512, (th + 1) * 512)
                g_ps = psc.tile([E, 512], F32, tag="sm", name=f"g{th}")
                nc.tensor.matmul(g_ps, g2_sb, hn_bf[:, sl], start=True, stop=True)
                nc.vector.tensor_scalar(
                    out=gates[:, sl], in0=g_ps, scalar1=gb2c_sb,
                    scalar2=None, op0=ALU.add,
                )

            # ---- top-2 routing, expert-major ------------------------------
            v1 = rt.tile([E, TPC], F32, tag="v1")
            nc.gpsimd.partition_all_reduce(v1, gates, channels=E, reduce_op=RED.max)
            oh1 = rt.tile([E, TPC], F32, tag="oh1")
            nc.vector.tensor_tensor(out=oh1, in0=gates, in1=v1, op=ALU.is_ge)
            msk = rt.tile([E, TPC], F32, tag="msk")
            nc.vector.scalar_tensor_tensor(
                out=msk, in0=oh1, scalar=NEG, in1=gates, op0=ALU.mult, op1=ALU.add
            )
            v2 = rt.tile([E, TPC], F32, tag="v2")
            nc.gpsimd.partition_all_reduce(v2, msk, channels=E, reduce_op=RED.max)
            oh2 = rt.tile([E, TPC], F32, tag="oh2")
            nc.vector.tensor_tensor(out=oh2, in0=msk, in1=v2, op=ALU.is_ge)
            nc.vector.tensor_tensor(out=msk, in0=v1, in1=v2, op=ALU.subtract)
            s1 = rt.tile([E, TPC], F32, tag="s1")
            nc.scalar.activation(s1, msk, ACT.Sigmoid)
            u1 = rt.tile([E, TPC], F32, tag="u1")
            nc.vector.tensor_tensor(out=u1, in0=oh1, in1=s1, op=ALU.mult)
            u2 = rt.tile([E, TPC], F32, tag="u2")
            # u2 = oh2 * (1 - s1)
            nc.vector.scalar_tensor_tensor(
                out=u2, in0=s1, scalar=-1.0, in1=oh2, op0=ALU.mult, op1=ALU.add
            )
            nc.vector.tensor_tensor(out=u2, in0=u2, in1=oh2, op=ALU.mult)
            cnt = rt.tile([E, 2], F32, tag="cnt")
            nc.vector.tensor_reduce(out=cnt[:, 0:1], in_=oh1, axis=AX.X, op=ALU.add)
            nc.vector.tensor_reduce(out=cnt[:, 1:2], in_=oh2, axis=AX.X, op=ALU.add)
            cc_in = dp.tile([E, 2], F32)
            cc_out = dp.tile([E, 2], F32)
            nc.gpsimd.dma_start(cc_in, cnt)
            nc.gpsimd.collective_compute(
                "AllReduce",
                ALU.add,
                replica_groups=[list(range(N_CORES))],
                ins=[cc_in.opt()],
                outs=[cc_out.opt()],
            )
            cntg = rt.tile([E, 2], F32, tag="cntg")
            nc.gpsimd.dma_start(cntg, cc_out)

            # ---- unfused main blocks while the collective runs ------------
            for oc in range(4, FUSE_OC):
                main_oc(oc, False)

            # ---- post-collective combine (vector queue tail) --------------
            alw = rt.tile([E, 2], F32, tag="alw")
            nc.vector.tensor_scalar(
                out=alw, in0=cntg, scalar1=CAPACITY + 0.5, scalar2=None,
                op0=ALU.is_le,
            )
            q2 = rt.tile([E, TPC], F32, tag="q2")
            nc.vector.tensor_scalar(
                out=q2, in0=u2, scalar1=alw[:, 1:2], scalar2=None, op0=ALU.mult
            )
            w_bf = big.tile([E, TPC], BF16)
            nc.vector.scalar_tensor_tensor(
                out=w_bf, in0=u1, scalar=alw[:, 0:1], in1=q2,
                op0=ALU.mult, op1=ALU.add,
            )
            for th in range(TH):
                sl = slice(th * 512, (th + 1) * 512)
                wbr = psc.tile([128, 512], F32, tag="sm", name=f"wbr{th}")
                nc.tensor.matmul(wbr, repp_sb, w_bf[:, sl], start=True, stop=True)
                nc.vector.tensor_tensor(
                    out=tw_bf[:, sl], in0=tmp_sb[:, sl], in1=wbr, op=ALU.mult
                )

            # ---- fused main blocks ----------------------------------------
            for oc in range(FUSE_OC, OC):
                main_oc(oc, True)

            # ---- LoRA tail for the unfused blocks -------------------------
            for oc in range(FUSE_OC):
                for th in range(TH):
                    sl = slice(th * 512, (th + 1) * 512)
                    pool = psm if (oc * TH + th) % 2 == 0 else psc
                    tag = "po" if pool is psm else "sm"
                    lp = pool.tile([128, 512], F32, tag=tag, name=f"lp{oc}_{th}")
                    nc.tensor.matmul(
                        lp, b_sb[:, oc * 128 : (oc + 1) * 128], tw_bf[:, sl],
                        start=True, stop=True,
                    )
                    lsb = op_.tile([128, 512], F32, tag="lsb")
                    if th == 0:
                        nc.scalar.activation(lsb, lp, ACT.Copy)
                    else:
                        nc.vector.tensor_copy(lsb, lp)
                    nc.sync.dma_start(
                        loraT.ap()[oc * 128 : (oc + 1) * 128, sl], lsb
                    )
    return nc


_CACHE = {}


def _get_nc():
    if "nc" not in _CACHE:
        nc = build_bass()
        nc.finalize()
        _CACHE["nc"] = nc
    return _CACHE["nc"]


def prep_in_maps(inputs):
    x = np.asarray(inputs["x"], dtype=np.float32)
    weight = np.asarray(inputs["weight"], dtype=np.float32)
    xf = x.reshape(N_TOK, IN)
    # wp[oc, p, k*128+c] = weight[oc*128+c, k*128+p]
    wp = np.ascontiguousarray(
        weight.reshape(OC, 128, KT, 128).transpose(0, 3, 2, 1).reshape(OC, 128, KT * 128)
    ).astype(BF)
    a_cat = (
        np.asarray(inputs["lora_A"], np.float32).transpose(1, 0, 2).reshape(IN, ER)
        * ALPHA
    )
    ap_ = np.ascontiguousarray(
        a_cat.reshape(KT, 128, ER).transpose(1, 0, 2).reshape(128, KT * ER)
    ).astype(BF)
    # centered gate weights: LN mean subtraction folded into G1 and gb1
    g1T = np.asarray(inputs["gw1"], np.float32).T  # [IN, 32]
    g1T = g1T - g1T.mean(axis=1, keepdims=True)
    g1p = np.ascontiguousarray(
        g1T.reshape(KT, 128, G4E).transpose(1, 0, 2).reshape(128, KT * G4E)
    ).astype(BF)
    gb1 = np.asarray(inputs["gb1"], np.float32)
    gb1 = gb1 - gb1.mean()
    bp = np.asarray(inputs["lora_B"], np.float32).reshape(ER, OUT).astype(BF)
    g2p = np.ascontiguousarray(np.asarray(inputs["gw2"], np.float32).T).astype(BF)
    repm = np.zeros((E, ER), np.float32)
    for e in range(E):
        repm[e, e * R : (e + 1) * R] = 1.0
    repp = repm.astype(BF)
    gb1c = np.ascontiguousarray(gb1.reshape(G4E, 1))
    gamc = np.ascontiguousarray(
        np.asarray(inputs["ln_gamma"], np.float32).reshape(G4E, 1)
    )
    betc = np.ascontiguousarray(
        np.asarray(inputs["ln_beta"], np.float32).reshape(G4E, 1)
    )
    gb2c = np.ascontiguousarray(np.asarray(inputs["gb2"], np.float32).reshape(E, 1))

    shared = dict(
        wp=wp, ap_=ap_, g1p=g1p, bp=bp, g2p=g2p, repp=repp,
        gb1c=gb1c, gamc=gamc, betc=betc, gb2c=gb2c,
    )
    in_maps = []
    for c in range(N_CORES):
        xs = xf[c * TPC : (c + 1) * TPC]  # [TPC, IN]
        xpc = np.ascontiguousarray(
            xs.T.reshape(KT, 128, TPC).transpose(1, 0, 2).reshape(128, KT * TPC)
        ).astype(BF)
        in_maps.append(dict(xp=xpc, **shared))
    return in_maps


def gather(results):
    out = np.empty((N_TOK, OUT), np.float32)
    for c in range(N_CORES):
        tot = np.array(results[c]["outT"])
        tot[: FUSE_OC * 128] += results[c]["loraT"]
        out[c * TPC : (c + 1) * TPC] = tot.T
    return out.reshape(B, S, OUT)


def kernel(**inputs):
    in_maps = prep_in_maps(inputs)
    nc = _get_nc()
    res = run_bass_kernel_spmd(nc, in_maps, core_ids=list(range(N_CORES)))
    return gather(res.results)
